# revision 9
# baseline (speedup 1.0000x reference)
"""Trainium2 Bass kernel for a 6+6 layer encoder-decoder transformer
(post-LN, 12 heads, D=768, F=3072, V=32000, B=8, S=512).

Sharding: pure data-parallel over batch — each of the 8 NeuronCores runs one
full sequence end-to-end (no collectives). Activations live feature-major
([feature partitions, token free-dim]); per-token statistics (LN mean/var,
softmax denominators) are produced as partition-replicated tiles via matmuls
against constant stationary operands so they can be consumed elementwise.
All matmuls run in float32r (1-pass FP22 multiply, fp32 storage/accum).
"""

import os
import sys

sys.path.insert(0, "/opt/trn_rl_repo")

import numpy as np
from contextlib import ExitStack

import concourse.bass as bass
import concourse.tile as tile
from concourse import bacc, mybir
from concourse import bass_utils
from concourse.masks import make_identity

F32 = mybir.dt.float32
F32R = mybir.dt.float32r
I32 = mybir.dt.int32
AF = mybir.ActivationFunctionType
OP = mybir.AluOpType

P = 128
T = 512            # tokens per core (one sequence)
D = 768
KD = D // P        # 6 feature tiles
H = 12
HD = 64
F = 3072
MF = F // P        # 24
V = 32000
L = 6
NCORES = 8
TT4 = T // P       # 4 token tiles
EPS = 1e-12

VCH = [(i * 512, 512) for i in range(V // 512)]
if V % 512:
    VCH.append((V - V % 512, V % 512))


def _r(ap):
    return ap if ap.dtype == F32R else ap.bitcast(F32R)


def _f(ap):
    return ap.bitcast(F32) if ap.dtype == F32R else ap


class Builder:
    def __init__(self, ctx, tc):
        self.ctx = ctx
        self.tc = tc
        self.nc = tc.nc
        pool = lambda name, bufs, space="SBUF": ctx.enter_context(
            tc.tile_pool(name=name, bufs=bufs, space=space))
        self.consts = pool("consts", 1)
        self.bpool = pool("biases", 2)       # small cols / masks / ids
        self.xpool = pool("acts", 1)         # persistent activations
        self.wpool = pool("weights", 6)      # [128,768] weight k-tiles
        self.lnpool = pool("lnscratch", 1)   # LN scratch [128,512]
        self._uid = 0

        nc = self.nc
        idf = self.consts.tile([P, P], F32, tag="idf")
        make_identity(nc, idf[:])
        self.ident_f = idf
        idr = self.consts.tile([P, P], F32R, tag="idr")
        nc.vector.tensor_copy(idr[:], idf[:])
        self.ident_r = idr
        sw = self.consts.tile([P, P], F32, tag="swf")
        nc.gpsimd.memset(sw[:], 1.0 / D)
        swr = self.consts.tile([P, P], F32R, tag="swr")
        nc.vector.tensor_copy(swr[:], sw[:])
        self.sumw_r = swr                     # [128,128] of 1/768
        on = self.consts.tile([P, HD], F32, tag="onf")
        nc.gpsimd.memset(on[:], 1.0)
        onr = self.consts.tile([P, HD], F32R, tag="onr")
        nc.vector.tensor_copy(onr[:], on[:])
        self.ones64_r = onr                   # [128,64] ones
        o1 = self.consts.tile([1, P], F32, tag="o1f")
        nc.gpsimd.memset(o1[:], 1.0)
        o1r = self.consts.tile([1, P], F32R, tag="o1r")
        nc.vector.tensor_copy(o1r[:], o1[:])
        self.ones_row_r = o1r                 # [1,128] ones
        ep = self.consts.tile([P, 1], F32, tag="epsb")
        nc.gpsimd.memset(ep[:], float(EPS))
        self.eps_tile = ep
        nb = self.consts.tile([P, 1], F32, tag="negb")
        nc.gpsimd.memset(nb[:], -10000.0)
        self.neg_tile = nb

    def uid(self):
        self._uid += 1
        return self._uid

    # ---------- small loads ----------

    def load_col(self, dram_row_ap, n, tag):
        """[n*128] dram vector -> [128, n] sbuf col tile (f32)."""
        t = self.bpool.tile([P, n], F32, tag=tag)
        self.nc.sync.dma_start(
            t[:], dram_row_ap.rearrange("(a p) -> p a", p=P))
        return t

    def load_wmat(self, dram_ap, tag="wmat"):
        out = []
        for k in range(KD):
            w = self.wpool.tile([P, D], F32R, tag=tag)
            self.nc.sync.dma_start(w[:], _r(dram_ap[k * P:(k + 1) * P, :]))
            out.append(w)
        return out

    def mask_bias(self, mask_dram, tag):
        nc = self.nc
        mi = self.bpool.tile([P, TT4], I32, tag=f"{tag}i")
        nc.sync.dma_start(mi[:], mask_dram[:])
        mf = self.bpool.tile([P, TT4], F32, tag=f"{tag}f")
        nc.vector.tensor_copy(mf[:], mi[:])
        mb = self.bpool.tile([P, TT4], F32, tag=f"{tag}b")
        nc.scalar.activation(mb[:], mf[:], AF.Identity,
                             bias=self.neg_tile[:], scale=10000.0)
        return mb

    # ---------- building blocks ----------

    def layernorm(self, y, w_col, b_col, out_fam):
        """Feature-major LN with partition-replicated stats (matmul sums)."""
        nc = self.nc
        sctx = ExitStack()
        lps = sctx.enter_context(
            self.tc.tile_pool(name="lnps", bufs=1, space="PSUM"))
        ps_mu = lps.tile([P, T], F32, tag="l0")
        ps_sq = lps.tile([P, T], F32, tag="l1")
        for t in range(KD):
            sq = self.lnpool.tile([P, T], F32R, tag=f"lnsq{t % 2}")
            nc.scalar.activation(sq[:], _f(y[t][:]), AF.Square)
            nc.tensor.matmul(ps_mu[:], self.sumw_r[:], y[t][:],
                             start=(t == 0), stop=(t == KD - 1))
            nc.tensor.matmul(ps_sq[:], self.sumw_r[:], sq[:],
                             start=(t == 0), stop=(t == KD - 1))
        mu = self.lnpool.tile([P, T], F32, tag="lnmu")
        nc.scalar.copy(mu[:], ps_mu[:])
        mu2 = self.lnpool.tile([P, T], F32, tag="lnmu2")
        nc.vector.tensor_tensor(out=mu2[:], in0=mu[:], in1=mu[:], op=OP.mult)
        var = self.lnpool.tile([P, T], F32, tag="lnvar")
        nc.vector.tensor_tensor(out=var[:], in0=ps_sq[:], in1=mu2[:],
                                op=OP.subtract)
        sctx.close()
        lnv = self.lnpool.tile([P, T], F32, tag="lnlog")
        nc.scalar.activation(lnv[:], var[:], AF.Ln, bias=self.eps_tile[:])
        rstd = self.lnpool.tile([P, T], F32, tag="lnrstd")
        nc.scalar.activation(rstd[:], lnv[:], AF.Exp, bias=0.0, scale=-0.5)
        out = []
        for t in range(KD):
            xc = self.lnpool.tile([P, T], F32, tag=f"lnxc{t % 2}")
            nc.vector.tensor_tensor(out=xc[:], in0=_f(y[t][:]), in1=mu[:],
                                    op=OP.subtract)
            xn = self.lnpool.tile([P, T], F32, tag=f"lnxn{t % 2}")
            nc.vector.tensor_tensor(out=xn[:], in0=xc[:], in1=rstd[:],
                                    op=OP.mult)
            o = self.xpool.tile([P, T], F32R, tag=f"{out_fam}{t}")
            nc.scalar.activation(o[:], xn[:], AF.Identity,
                                 bias=b_col[:, t:t + 1],
                                 scale=w_col[:, t:t + 1])
            out.append(o)
        return out

    def attention(self, xq, xkv, mb, wq_d, bq_d, wk_d, bk_d, wv_d, bv_d,
                  wo_d, bo_d):
        """MHA + residual; returns pre-LN tiles in global 'ao' tags."""
        nc = self.nc
        with ExitStack() as sctx:
            spool = lambda name, bufs, space="SBUF": sctx.enter_context(
                self.tc.tile_pool(name=name, bufs=bufs, space=space))
            qkv = spool("qkv", 1)
            hpool = spool("esc", 5)
            cpool = spool("ctxh", 13)
            rpool = spool("recs", 2)
            aps = spool("attnps", 3, "PSUM")
            cps = spool("cps", 1, "PSUM")
            dps = spool("dps", 1, "PSUM")

            def linear(x, w_tiles, bias_col, fam):
                outl = []
                for m in range(KD):
                    ps = aps.tile([P, T], F32, tag="aps")
                    for k in range(KD):
                        nc.tensor.matmul(ps[:],
                                         w_tiles[k][:, m * P:(m + 1) * P],
                                         x[k][:], start=(k == 0),
                                         stop=(k == KD - 1))
                    o = qkv.tile([P, T], F32R, tag=f"{fam}{m}")
                    nc.scalar.activation(o[:], ps[:], AF.Identity,
                                         bias=bias_col[:, m:m + 1], scale=1.0)
                    outl.append(o)
                return outl

            wq = self.load_wmat(wq_d)
            bq = self.load_col(bq_d, KD, "bqc")
            q = linear(xq, wq, bq, "q")
            wk = self.load_wmat(wk_d)
            bk = self.load_col(bk_d, KD, "bkc")
            k = linear(xkv, wk, bk, "k")
            wv = self.load_wmat(wv_d)
            bv_row = self.bpool.tile([1, D], F32R, tag="bvrow")
            nc.sync.dma_start(bv_row[:], _r(bv_d[None, :]))
            v = []
            for tm in range(TT4):
                vt = qkv.tile([P, D], F32R, tag=f"v{tm}")
                for half in range(2):
                    ps = aps.tile([P, T], F32, tag="aps")
                    cs = slice(half * 384, (half + 1) * 384)
                    for kk in range(KD):
                        nc.tensor.matmul(ps[:, :384],
                                         xkv[kk][:, tm * P:(tm + 1) * P],
                                         wv[kk][:, cs], start=(kk == 0),
                                         stop=False)
                    nc.tensor.matmul(ps[:, :384], self.ones_row_r[:],
                                     bv_row[:, cs], start=False, stop=True)
                    nc.scalar.copy(vt[:, cs], ps[:, :384])
                v.append(vt)

            ctxs = []
            for h in range(H):
                kt, off = divmod(h * HD, P)
                es = []
                for kb in range(TT4):
                    ps = aps.tile([P, T], F32, tag="aps")
                    nc.tensor.matmul(
                        ps[:], k[kt][off:off + HD, kb * P:(kb + 1) * P],
                        q[kt][off:off + HD, :], start=True, stop=True)
                    e = hpool.tile([P, T], F32R, tag="esc")
                    nc.scalar.activation(e[:], ps[:], AF.Exp,
                                         bias=mb[:, kb:kb + 1], scale=0.125)
                    es.append(e)
                ps_c = cps.tile([HD, T], F32, tag="cps")
                ps_d = dps.tile([HD, T], F32, tag="dps")
                for kb in range(TT4):
                    nc.tensor.matmul(ps_c[:], v[kb][:, h * HD:(h + 1) * HD],
                                     es[kb][:], start=(kb == 0),
                                     stop=(kb == TT4 - 1))
                    nc.tensor.matmul(ps_d[:], self.ones64_r[:], es[kb][:],
                                     start=(kb == 0), stop=(kb == TT4 - 1))
                rec = rpool.tile([HD, T], F32, tag="recd")
                nc.vector.reciprocal_approx_fast(rec[:], ps_d[:])
                ch = cpool.tile([HD, T], F32R, tag="ctxh")
                nc.vector.tensor_tensor(out=ch[:], in0=ps_c[:], in1=rec[:],
                                        op=OP.mult)
                ctxs.append(ch)

            woh = spool("woh", 16)
            bo = self.load_col(bo_d, KD, "boc")
            out = []
            for m in range(KD):
                ps = aps.tile([P, T], F32, tag="aps")
                nc.tensor.matmul(ps[:], self.ident_r[:], xq[m][:],
                                 start=True, stop=False)
                for h in range(H):
                    w = woh.tile([HD, P], F32R, tag="woh")
                    nc.sync.dma_start(
                        w[:], _r(wo_d[h * HD:(h + 1) * HD,
                                      m * P:(m + 1) * P]))
                    nc.tensor.matmul(
                        ps[:], w[:], ctxs[h][:], start=False,
                        stop=(h == H - 1))
                o = self.xpool.tile([P, T], F32R, tag=f"po{m}")
                nc.scalar.activation(o[:], ps[:], AF.Identity,
                                     bias=bo[:, m:m + 1], scale=1.0)
                out.append(o)
            return out

    def ffn(self, x, w1t_d, b1_d, w2_d, b2_d):
        """FFN + residual. w1t_d is host-pretiled [24, 128, 768]
        (m1, k-partition, (k, m)). Returns pre-LN tiles in 'fo' tags."""
        nc = self.nc
        with ExitStack() as sctx:
            spool = lambda name, bufs, space="SBUF": sctx.enter_context(
                self.tc.tile_pool(name=name, bufs=bufs, space=space))
            wf = spool("wffn", 3)
            gp = spool("gelu", 3)
            fps = sctx.enter_context(
                self.tc.tile_pool(name="fps", bufs=KD, space="PSUM"))
            f1ps = sctx.enter_context(
                self.tc.tile_pool(name="f1ps", bufs=2, space="PSUM"))
            b1 = self.load_col(b1_d, MF, "b1c")
            ps2 = [fps.tile([P, T], F32, tag="fps", name=f"fps{_i}")
                   for _i in range(KD)]
            for m2 in range(KD):
                nc.tensor.matmul(ps2[m2][:], self.ident_r[:], x[m2][:],
                                 start=True, stop=False)
            for m1 in range(MF):
                w1t = wf.tile([P, D], F32R, tag="w1t")
                nc.sync.dma_start(w1t[:], _r(w1t_d[m1]))
                ps = f1ps.tile([P, T], F32, tag="f1")
                for kk in range(KD):
                    nc.tensor.matmul(ps[:], w1t[:, kk * P:(kk + 1) * P],
                                     x[kk][:], start=(kk == 0),
                                     stop=(kk == KD - 1))
                g = gp.tile([P, T], F32R, tag="gelu")
                gfunc = AF.Tanh if os.environ.get("BTK_SIMSAFE") else AF.Gelu
                nc.scalar.activation(g[:], ps[:], gfunc,
                                     bias=b1[:, m1:m1 + 1], scale=1.0)
                w2t = wf.tile([P, D], F32R, tag="w2t")
                nc.sync.dma_start(w2t[:], _r(w2_d[m1 * P:(m1 + 1) * P, :]))
                for m2 in range(KD):
                    nc.tensor.matmul(ps2[m2][:], w2t[:, m2 * P:(m2 + 1) * P],
                                     g[:], start=False, stop=(m1 == MF - 1))
            b2 = self.load_col(b2_d, KD, "b2c")
            out = []
            for m2 in range(KD):
                o = self.xpool.tile([P, T], F32R, tag=f"po{m2}")
                nc.scalar.activation(o[:], ps2[m2][:], AF.Identity,
                                     bias=b2[:, m2:m2 + 1], scale=1.0)
                out.append(o)
            return out

    def embed(self, ids_sb, tok_emb_d, pos_emb_d):
        """Gather + positional add, transposed to feature-major ('ao' tags)."""
        nc = self.nc
        with ExitStack() as sctx:
            ep = sctx.enter_context(self.tc.tile_pool(name="embp", bufs=2))
            eps = sctx.enter_context(
                self.tc.tile_pool(name="embps", bufs=2, space="PSUM"))
            tm_tiles = []
            for tm in range(TT4):
                em = ep.tile([P, D], F32, tag="embg")
                nc.gpsimd.indirect_dma_start(
                    out=em[:], out_offset=None, in_=tok_emb_d[:],
                    in_offset=bass.IndirectOffsetOnAxis(
                        ap=ids_sb[:, tm:tm + 1], axis=0))
                po = ep.tile([P, D], F32, tag="embp")
                nc.sync.dma_start(po[:], pos_emb_d[tm * P:(tm + 1) * P, :])
                e2 = ep.tile([P, D], F32, tag="embs", bufs=4)
                nc.vector.tensor_tensor(out=e2[:], in0=em[:], in1=po[:],
                                        op=OP.add)
                tm_tiles.append(e2)
            out = []
            for t in range(KD):
                o = self.xpool.tile([P, T], F32R, tag=f"po{t}")
                for tm in range(TT4):
                    ps = eps.tile([P, P], F32, tag="etr")
                    nc.tensor.transpose(ps[:],
                                        tm_tiles[tm][:, t * P:(t + 1) * P],
                                        self.ident_f[:])
                    nc.scalar.copy(o[:, tm * P:(tm + 1) * P], ps[:])
                out.append(o)
            return out

    def vocab_proj(self, y, outw_d, outb_d, logits, vch):
        nc = self.nc
        with ExitStack() as sctx:
            spool = lambda name, bufs, space="SBUF": sctx.enter_context(
                self.tc.tile_pool(name=name, bufs=bufs, space=space))
            wv = spool("wvoc", 8)
            lsb = spool("lsb", 4)
            vps = spool("vps", 3, "PSUM")
            for (n0, nsz) in vch:
                psb = vps.tile([P, 512], F32, tag="vob")
                ob_row = self.bpool.tile([1, 512], F32R, tag="obrow")
                nc.sync.dma_start(ob_row[:, :nsz],
                                  _r(outb_d[None, n0:n0 + nsz]))
                nc.tensor.matmul(psb[:, :nsz], self.ones_row_r[:],
                                 ob_row[:, :nsz], start=True, stop=True)
                obs = lsb.tile([P, 512], F32, tag="obsb")
                nc.scalar.copy(obs[:, :nsz], psb[:, :nsz])
                wct = []
                for kk in range(KD):
                    w = wv.tile([P, 512], F32R, tag="wvoc")
                    nc.sync.dma_start(
                        w[:, :nsz], _r(outw_d[kk * P:(kk + 1) * P,
                                              n0:n0 + nsz]))
                    wct.append(w)
                for tm in range(TT4):
                    ps = vps.tile([P, 512], F32, tag="vps")
                    for kk in range(KD):
                        nc.tensor.matmul(ps[:, :nsz],
                                         y[kk][:, tm * P:(tm + 1) * P],
                                         wct[kk][:, :nsz], start=(kk == 0),
                                         stop=(kk == KD - 1))
                    ls = lsb.tile([P, 512], F32, tag="lgsb")
                    nc.vector.tensor_tensor(out=ls[:, :nsz], in0=ps[:, :nsz],
                                            in1=obs[:, :nsz], op=OP.add)
                    nc.sync.dma_start(
                        logits[tm * P:(tm + 1) * P, n0:n0 + nsz],
                        ls[:, :nsz])


def build_program(l_enc=None, l_dec=None, nvch=None):
    l_enc = int(os.environ.get("BTK_LENC", L)) if l_enc is None else l_enc
    l_dec = int(os.environ.get("BTK_LDEC", L)) if l_dec is None else l_dec
    nvch = int(os.environ.get("BTK_NVCH", len(VCH))) if nvch is None else nvch
    vch = VCH[:nvch]

    nc = bacc.Bacc("TRN2", target_bir_lowering=False, debug=False,
                   num_devices=NCORES)
    dt = {}

    def dram(name, shape, dtype=F32, kind="ExternalInput"):
        dt[name] = nc.dram_tensor(name, list(shape), dtype, kind=kind).ap()
        return dt[name]

    dram("ids_e", (P, TT4), I32)
    dram("ids_d", (P, TT4), I32)
    dram("mask_e", (P, TT4), I32)
    dram("mask_d", (P, TT4), I32)
    dram("tok_emb", (V, D))
    dram("pos_emb", (T, D))
    dram("lnw0", (D,))
    dram("lnb0", (D,))
    for pfx, nl in (("e", l_enc), ("d", l_dec)):
        mats = (["wq", "wk", "wv", "wo"] if pfx == "e"
                else ["swq", "swk", "swv", "swo", "cwq", "cwk", "cwv", "cwo"])
        for nm in mats:
            dram(f"{pfx}_{nm}", (nl, D, D))
            dram(f"{pfx}_{nm}_b", (nl, D))
        dram(f"{pfx}_w1t", (nl, MF, P, D))   # host-pretiled
        dram(f"{pfx}_b1", (nl, F))
        dram(f"{pfx}_w2", (nl, F, D))
        dram(f"{pfx}_b2", (nl, D))
        for i in range(1, (3 if pfx == "e" else 4)):
            dram(f"{pfx}_ln{i}w", (nl, D))
            dram(f"{pfx}_ln{i}b", (nl, D))
    dram("out_w", (D, V))
    dram("out_b", (V,))
    logits = dram("logits", (T, V), F32, kind="ExternalOutput")

    with ExitStack() as ctx:
        tc = ctx.enter_context(tile.TileContext(nc))
        b = Builder(ctx, tc)
        nc_ = b.nc

        ids_e = b.bpool.tile([P, TT4], I32, tag="idse")
        nc_.sync.dma_start(ids_e[:], dt["ids_e"][:])
        ids_d = b.bpool.tile([P, TT4], I32, tag="idsd")
        nc_.sync.dma_start(ids_d[:], dt["ids_d"][:])
        mbe = b.mask_bias(dt["mask_e"], "me")
        mbd = b.mask_bias(dt["mask_d"], "md")
        lnw0 = b.load_col(dt["lnw0"], KD, "lnw0")
        lnb0 = b.load_col(dt["lnb0"], KD, "lnb0")

        fams = ["xa", "xb"]
        fi = 0

        def next_fam():
            nonlocal fi
            fam = fams[fi % 2]
            fi += 1
            return fam

        # ---------------- encoder ----------------
        h0 = b.embed(ids_e, dt["tok_emb"], dt["pos_emb"])
        x = b.layernorm(h0, lnw0, lnb0, next_fam())
        for l in range(l_enc):
            a = b.attention(x, x, mbe,
                            dt["e_wq"][l], dt["e_wq_b"][l],
                            dt["e_wk"][l], dt["e_wk_b"][l],
                            dt["e_wv"][l], dt["e_wv_b"][l],
                            dt["e_wo"][l], dt["e_wo_b"][l])
            x = b.layernorm(a, b.load_col(dt["e_ln1w"][l], KD, "lw1"),
                            b.load_col(dt["e_ln1b"][l], KD, "lb1"),
                            next_fam())
            ff = b.ffn(x, dt["e_w1t"][l], dt["e_b1"][l],
                       dt["e_w2"][l], dt["e_b2"][l])
            fam = "enc" if l == l_enc - 1 else next_fam()
            x = b.layernorm(ff, b.load_col(dt["e_ln2w"][l], KD, "lw2"),
                            b.load_col(dt["e_ln2b"][l], KD, "lb2"), fam)
        enc_out = x

        # ---------------- decoder ----------------
        g0 = b.embed(ids_d, dt["tok_emb"], dt["pos_emb"])
        y = b.layernorm(g0, lnw0, lnb0, next_fam())
        for l in range(l_dec):
            a = b.attention(y, y, mbd,
                            dt["d_swq"][l], dt["d_swq_b"][l],
                            dt["d_swk"][l], dt["d_swk_b"][l],
                            dt["d_swv"][l], dt["d_swv_b"][l],
                            dt["d_swo"][l], dt["d_swo_b"][l])
            y = b.layernorm(a, b.load_col(dt["d_ln1w"][l], KD, "lw1"),
                            b.load_col(dt["d_ln1b"][l], KD, "lb1"),
                            next_fam())
            c = b.attention(y, enc_out, mbe,
                            dt["d_cwq"][l], dt["d_cwq_b"][l],
                            dt["d_cwk"][l], dt["d_cwk_b"][l],
                            dt["d_cwv"][l], dt["d_cwv_b"][l],
                            dt["d_cwo"][l], dt["d_cwo_b"][l])
            y = b.layernorm(c, b.load_col(dt["d_ln2w"][l], KD, "lw2"),
                            b.load_col(dt["d_ln2b"][l], KD, "lb2"),
                            next_fam())
            ff = b.ffn(y, dt["d_w1t"][l], dt["d_b1"][l],
                       dt["d_w2"][l], dt["d_b2"][l])
            y = b.layernorm(ff, b.load_col(dt["d_ln3w"][l], KD, "lw3"),
                            b.load_col(dt["d_ln3b"][l], KD, "lb3"),
                            next_fam())

        b.vocab_proj(y, dt["out_w"], dt["out_b"], logits, vch)

    nc.compile()
    return nc


_CACHE = {}


def _get_program():
    if "nc" not in _CACHE:
        _CACHE["nc"] = build_program()
    return _CACHE["nc"]


def _rearr_ids(a):
    return np.ascontiguousarray(
        np.asarray(a).reshape(TT4, P).T).astype(np.int32)


def _tile_w1(w1):
    # [L, D, F] -> [L, MF, P, D]: w1t[l, m1, p, k*128+m] = w1[l, k*128+p, m1*128+m]
    L_, D_, F_ = w1.shape
    t = w1.reshape(L_, KD, P, MF, P).transpose(0, 3, 2, 1, 4)
    return np.ascontiguousarray(t.reshape(L_, MF, P, D_))


def kernel(params, input_ids, attention_mask, labels, decoder_attention_mask):
    np_ = lambda a: np.ascontiguousarray(np.asarray(a), dtype=np.float32)
    nc = _get_program()

    enc, dec = params["enc"], params["dec"]
    shared = {
        "tok_emb": np_(params["tok_emb"]),
        "pos_emb": np_(params["pos_emb"][:T]),
        "lnw0": np_(params["ln_w"]),
        "lnb0": np_(params["ln_b"]),
        "out_w": np_(params["out_w"]),
        "out_b": np_(params["out_b"]),
    }
    for key in ("wq", "wk", "wv", "wo"):
        shared[f"e_{key}"] = np_(enc[key])
        shared[f"e_{key}_b"] = np_(enc["b" + key[1]])
    shared["e_w1t"] = _tile_w1(np_(enc["w1"]))
    shared["e_b1"] = np_(enc["b1"])
    shared["e_w2"], shared["e_b2"] = np_(enc["w2"]), np_(enc["b2"])
    for i in (1, 2):
        shared[f"e_ln{i}w"] = np_(enc[f"ln{i}_w"])
        shared[f"e_ln{i}b"] = np_(enc[f"ln{i}_b"])
    for nm in ("swq", "swk", "swv", "swo", "cwq", "cwk", "cwv", "cwo"):
        shared[f"d_{nm}"] = np_(dec[nm[0] + "_w" + nm[-1]])
        shared[f"d_{nm}_b"] = np_(dec[nm[0] + "_b" + nm[-1]])
    shared["d_w1t"] = _tile_w1(np_(dec["w1"]))
    shared["d_b1"] = np_(dec["b1"])
    shared["d_w2"], shared["d_b2"] = np_(dec["w2"]), np_(dec["b2"])
    for i in (1, 2, 3):
        shared[f"d_ln{i}w"] = np_(dec[f"ln{i}_w"])
        shared[f"d_ln{i}b"] = np_(dec[f"ln{i}_b"])

    in_maps = []
    for c in range(NCORES):
        m = dict(shared)
        m["ids_e"] = _rearr_ids(input_ids[c])
        m["ids_d"] = _rearr_ids(labels[c])
        m["mask_e"] = _rearr_ids(attention_mask[c])
        m["mask_d"] = _rearr_ids(decoder_attention_mask[c])
        in_maps.append(m)

    res = bass_utils.run_bass_kernel_spmd(
        nc, in_maps, core_ids=list(range(NCORES)),
        trace=bool(os.environ.get("BTK_TRACE")))
    _CACHE["last"] = res
    return np.stack([res.results[c]["logits"] for c in range(NCORES)])
